# revision 1
# baseline (speedup 1.0000x reference)
import numpy as np

NBINS = 9
POOL = 7
PI = float(np.pi)


def _hog_shard(x, wx, wy):
    # x: [b,3,H,W] float32; wx/wy: [3,1,3,3] depthwise Sobel taps
    b, c, H, W = x.shape
    xp = np.pad(x, ((0, 0), (0, 0), (1, 1), (1, 1)), mode='reflect')
    gx = np.zeros_like(x)
    gy = np.zeros_like(x)
    for di in range(3):
        for dj in range(3):
            patch = xp[:, :, di:di + H, dj:dj + W]
            gx += patch * wx[:, 0, di, dj][None, :, None, None]
            gy += patch * wy[:, 0, di, dj][None, :, None, None]
    norm = np.sqrt(gx * gx + gy * gy)
    phase = np.arctan2(gx, gy) / PI * NBINS
    bins = np.mod(np.floor(phase).astype(np.int32), NBINS)
    Hp, Wp = H // POOL, W // POOL
    out = np.empty((b, c, NBINS, Hp, Wp), dtype=np.float32)
    for k in range(NBINS):
        m = np.where(bins == k, norm, np.float32(0.0))
        out[:, :, k] = m.reshape(b, c, Hp, POOL, Wp, POOL).sum(axis=(3, 5))
    denom = np.maximum(np.sqrt((out * out).sum(axis=2, keepdims=True)), 1e-12)
    return (out / denom).astype(np.float32)


def _try_device_path(x, wx, wy):
    import jax
    import jax.numpy as jnp
    devs = jax.devices()
    if len(devs) < 8:
        return None

    def per_shard(xs, w1, w2):
        xp = jnp.pad(xs, ((0, 0), (0, 0), (1, 1), (1, 1)), mode='reflect')
        def dw(w):
            return jax.lax.conv_general_dilated(
                xp, w, window_strides=(1, 1), padding='VALID',
                feature_group_count=3,
                dimension_numbers=('NCHW', 'OIHW', 'NCHW'))
        gx = dw(w1)
        gy = dw(w2)
        norm = jnp.sqrt(gx * gx + gy * gy)
        phase = jnp.arctan2(gx, gy) / PI * NBINS
        bins = jnp.mod(jnp.floor(phase).astype(jnp.int32), NBINS)
        b, c, H, W = norm.shape
        oh = jax.nn.one_hot(bins, NBINS, dtype=norm.dtype)
        out = jnp.moveaxis(oh * norm[..., None], -1, 2)
        out = out.reshape(b, c, NBINS, H // POOL, POOL, W // POOL, POOL).sum(axis=(4, 6))
        denom = jnp.maximum(jnp.linalg.norm(out, axis=2, keepdims=True), 1e-12)
        return out / denom

    pm = jax.pmap(per_shard, devices=devs[:8])
    B = x.shape[0]
    xs = x.reshape(8, B // 8, *x.shape[1:])
    w1 = np.broadcast_to(wx, (8,) + wx.shape)
    w2 = np.broadcast_to(wy, (8,) + wy.shape)
    out = np.asarray(pm(xs, w1, w2))
    return out.reshape(B, *out.shape[2:]).astype(np.float32)


def kernel(x, weight_x, weight_y):
    x = np.asarray(x, dtype=np.float32)
    wx = np.asarray(weight_x, dtype=np.float32)
    wy = np.asarray(weight_y, dtype=np.float32)
    try:
        out = _try_device_path(x, wx, wy)
        if out is not None:
            return out
    except Exception:
        pass
    return _hog_shard(x, wx, wy)
